# revision 61
# baseline (speedup 1.0000x reference)
"""Trainium2 Bass kernel for DiT focused-linear-attention block (nn_DiT_9259949490457).

Data-parallel over batch: 16 batches -> 8 NeuronCores, 2 batches/core, no collectives.

q and kv GEMMs run in fp8-e4m3 hi/lo split-precision using DoubleRow perf mode
(2 slice-products per 0.5-cycle/row instruction -> 14 DR units vs 18 bf16 units per
K=1152 contraction, with better-than-bf16 accuracy). The hi/lo fp8 cast of x is fused
into the transpose-psum evacuation (ACT writes hi, DVE subtract writes lo). The
depthwise 3x3 conv branch pairs (dy=-1,dy=+1) taps into DoubleRow fp8 matmuls via
custom overlapping access patterns.

The two local batches are phase-interleaved so PE always has matmul work while the
other batch's norm chain / einsum2-evacuation drains on ACT/DVE: x-prep stages feed
b0's K/V tiles just-in-time; b1's K fills b0's norm chain; b1's V/vT/e1 tiles are
emitted head-by-head as fillers inside b0's einsum2; b1's own proj tiles 0-3 fill
its einsum2 chunk 1; PSUM runs 4-deep on the GEMM tag with the norm row sharing
the dwconv/proj bank tag.

Self-contained: hardcodes all shapes; host numpy pre-packs fp8 hi/lo weights
(scaled by 64; 1/64 folded into psum-evacuation activations).
"""

import numpy as np
import ml_dtypes
import bass_rust

import concourse.bacc as bacc
import concourse.mybir as mybir
import concourse.tile as tile
from concourse import bass_utils

F32 = mybir.dt.float32
BF16 = mybir.dt.bfloat16
FP8 = mybir.dt.float8e4
ALU = mybir.AluOpType
AF = mybir.ActivationFunctionType
AX = mybir.AxisListType
DR = mybir.MatmulPerfMode.DoubleRow

NCORES = 8
B, N, DIM = 16, 1024, 1152
H, KVH, HD = 12, 4, 96
BL = B // NCORES          # 2 local batches
T = BL * N                # 2048 local tokens
NK = DIM // 128           # 9 feature K-slices
TT = N // 128             # 8 token tiles per batch
CH = N // 512             # 2 free-dim chunks of 512 per batch
SW = 64.0                 # fp8 weight pre-scale (power of two)

_BF = ml_dtypes.bfloat16
_F8 = ml_dtypes.float8_e4m3fn


def _spanp(b):
    if b % 128 == 0:
        return 128
    if b % 64 == 0:
        return 64
    return 32


def _head_pieces(h):
    out = []
    rr = 0
    while rr < 96:
        gr = 96 * h + rr
        j, r0 = divmod(gr, 128)
        cnt = min(96 - rr, 128 - r0, _spanp(r0), _spanp(rr))
        out.append((j, r0, rr, cnt))
        rr += cnt
    return out


def _ins_dim(ap, stride, count):
    """Insert a free dim [stride, count] right after the partition dim."""
    dims = [list(d) for d in ap.ap]
    new = [dims[0], [stride, count]] + dims[1:]
    return bass_rust.AP(ap.tensor, ap.offset, new)


class _St:
    pass


def _build_kernel():
    nc = bacc.Bacc("TRN2", target_bir_lowering=False, debug=False,
                   enable_asserts=True, num_devices=NCORES)
    x_in = nc.dram_tensor("x", [T, DIM], F32, kind="ExternalInput").ap()
    wqfj_in = nc.dram_tensor("wqfj", [128, NK, 2, NK, 128], FP8, kind="ExternalInput").ap()
    wq8r_in = nc.dram_tensor("wq8r", [128, 2, DIM], FP8, kind="ExternalInput").ap()
    wkvfk_in = nc.dram_tensor("wkvfk", [128, 2, NK, 384], FP8, kind="ExternalInput").ap()
    wkvfv_in = nc.dram_tensor("wkvfv", [128, 2, NK, 384], FP8, kind="ExternalInput").ap()
    wkv8rk_in = nc.dram_tensor("wkv8rk", [128, 2, 384], FP8, kind="ExternalInput").ap()
    wkv8rv_in = nc.dram_tensor("wkv8rv", [128, 2, 384], FP8, kind="ExternalInput").ap()
    pwT_in = nc.dram_tensor("pwT", [DIM, DIM], BF16, kind="ExternalInput").ap()
    wqb_in = nc.dram_tensor("wqb", [128, NK], F32, kind="ExternalInput").ap()
    kvb64_in = nc.dram_tensor("kvb64", [1, 768], BF16, kind="ExternalInput").ap()
    pjb_bc_in = nc.dram_tensor("pjb_bc", [128, DIM], BF16, kind="ExternalInput").ap()
    dgp_in = nc.dram_tensor("dgp", [96, 2, KVH, 3, 96], FP8, kind="ExternalInput").ap()
    dgz_in = nc.dram_tensor("dgz", [96, 2, KVH, 3, 96], FP8, kind="ExternalInput").ap()
    dge_in = nc.dram_tensor("dge", [96, KVH, 9, 96], FP8, kind="ExternalInput").ap()
    dwcb_in = nc.dram_tensor("dwcb", [96, KVH], F32, kind="ExternalInput").ap()
    masks_in = nc.dram_tensor("masks", [128, NK, H], BF16, kind="ExternalInput").ap()
    eye_in = nc.dram_tensor("eye", [128, 128], BF16, kind="ExternalInput").ap()
    y_out = nc.dram_tensor("y", [T, DIM], F32, kind="ExternalOutput").ap()

    from contextlib import ExitStack
    with tile.TileContext(nc) as tc, ExitStack() as stack:
        cpool = stack.enter_context(tc.tile_pool(name="const", bufs=1))
        dpool = stack.enter_context(tc.tile_pool(name="dram", bufs=1, space="DRAM"))

        # ---- consts / weights ----
        eye = cpool.tile([128, 128], BF16, name="eye")
        wkvfk = cpool.tile([128, 2, NK, 384], FP8, name="wkvfk")
        wkvfv = cpool.tile([128, 2, NK, 384], FP8, name="wkvfv")
        wkv8rk = cpool.tile([128, 2, 384], FP8, name="wkv8rk")
        wkv8rv = cpool.tile([128, 2, 384], FP8, name="wkv8rv")
        kvb64 = cpool.tile([1, 768], BF16, name="kvb64")
        wqfj = cpool.tile([128, NK, 2, NK, 128], FP8, name="wqfj")
        wq8r = cpool.tile([128, 2, DIM], FP8, name="wq8r")
        wqb = cpool.tile([128, NK], F32, name="wqb")
        PWT = [cpool.tile([128, DIM], BF16, name=f"PWT{k}") for k in range(NK)]
        pjb_bc = cpool.tile([128, DIM], BF16, name="pjb_bc")
        dgp = cpool.tile([96, 2, KVH, 3, 96], FP8, name="dgp")
        dgz = cpool.tile([96, 2, KVH, 3, 96], FP8, name="dgz")
        dge = cpool.tile([96, KVH, 9, 96], FP8, name="dge")
        dwcb = cpool.tile([96, KVH], F32, name="dwcb")
        masks = cpool.tile([128, NK, H], BF16, name="masks")
        ones_r = cpool.tile([1, 128], BF16, name="ones_r")
        ones_c = cpool.tile([128, 1], BF16, name="ones_c")
        sqwarm = cpool.tile([1, 8], F32, name="sqwarm")

        vpad = dpool.tile([BL, N, KVH, 128], BF16, name="vpad")

        # ---- pools ----
        xpool = stack.enter_context(tc.tile_pool(name="xf", bufs=1))
        XF = xpool.tile([128, 2, NK, T], FP8, name="XF")
        wp = stack.enter_context(tc.tile_pool(name="work", bufs=1))
        pmm = stack.enter_context(tc.tile_pool(name="pmm", bufs=1, space="PSUM"))
        pa = stack.enter_context(tc.tile_pool(name="pa", bufs=1, space="PSUM"))

        # ---- prologue machinery: load x, transpose on PE, evacuate as fp8
        # hi/lo into XF. Stages are interleaved with batch-0 K/V tiles so the
        # ACT/DVE evacuation queue never runs ahead of the GEMM consumers.
        prep = stack.enter_context(tc.tile_pool(name="prep", bufs=3))

        def prep_stage(i):
            stage = prep.tile([128, DIM], BF16, name="stage", tag="stage")
            nc.gpsimd.dma_start(out=stage[:],
                                in_=x_in[128 * i:128 * (i + 1), :])
            if i == 0:
                nc.sync.dma_start(out=eye[:], in_=eye_in[:])
                nc.vector.memset(ones_r[:], 1.0)
                nc.vector.memset(ones_c[:], 1.0)
                nc.vector.memset(sqwarm[:], 1.0)
                # warm the Sqrt activation table off the critical path
                nc.scalar.activation(sqwarm[:], sqwarm[:], AF.Sqrt)
            elif i == 1:
                nc.sync.dma_start(out=wkvfk[:], in_=wkvfk_in[:])
                nc.sync.dma_start(out=wkv8rk[:], in_=wkv8rk_in[:])
                nc.sync.dma_start(out=kvb64[:], in_=kvb64_in[:])
            elif i == 3:
                nc.sync.dma_start(out=wkvfv[:], in_=wkvfv_in[:])
                nc.sync.dma_start(out=wkv8rv[:], in_=wkv8rv_in[:])
            elif i == 5:
                for j in range(NK):
                    nc.sync.dma_start(out=wqfj[:, j], in_=wqfj_in[:, j])
                nc.sync.dma_start(out=wq8r[:], in_=wq8r_in[:])
                nc.sync.dma_start(out=wqb[:], in_=wqb_in[:])
            elif i == 9:
                for k in range(NK):
                    nc.sync.dma_start(out=PWT[k][:],
                                      in_=pwT_in[128 * k:128 * (k + 1), :])
                nc.sync.dma_start(out=pjb_bc[:], in_=pjb_bc_in[:])
            elif i == 11:
                nc.sync.dma_start(out=dgp[:], in_=dgp_in[:])
                nc.sync.dma_start(out=dgz[:], in_=dgz_in[:])
                nc.sync.dma_start(out=dge[:], in_=dge_in[:])
                nc.sync.dma_start(out=dwcb[:], in_=dwcb_in[:])
                nc.sync.dma_start(out=masks[:], in_=masks_in[:])
            pt_a = pa.tile([128, 1024], BF16, name="pt_a", tag="pa", bufs=2)
            for k in range(8):
                nc.tensor.transpose(pt_a[:, 128 * k:128 * (k + 1)],
                                    stage[:, 128 * k:128 * (k + 1)], eye[:])
            pt_b = pa.tile([128, 128], BF16, name="pt_b", tag="pa", bufs=2)
            nc.tensor.transpose(pt_b[:], stage[:, 1024:1152], eye[:])
            cs = slice(128 * i, 128 * (i + 1))
            pt3 = pt_a[:].rearrange("p (k c) -> p k c", c=128)
            # hi casts (ACT), slices 0-7 slot1, slice 8 slot0
            nc.scalar.activation(XF[:, 1, 0:8, cs], pt3, AF.Copy)
            nc.scalar.activation(XF[:, 0, 8, cs], pt_b[:], AF.Copy)
            # lo = psum - hi (DVE), slices 0-7 slot0, slice 8 slot1
            nc.vector.tensor_tensor(out=XF[:, 0, 0:8, cs], in0=pt3,
                                    in1=XF[:, 1, 0:8, cs], op=ALU.subtract)
            nc.vector.tensor_tensor(out=XF[:, 1, 8, cs], in0=pt_b[:],
                                    in1=XF[:, 0, 8, cs], op=ALU.subtract)

        cn = _St()
        cn.wqfj, cn.wq8r, cn.wqb = wqfj, wq8r, wqb
        cn.wkvfk, cn.wkvfv, cn.wkv8rk, cn.wkv8rv = wkvfk, wkvfv, wkv8rk, wkv8rv
        cn.kvb64, cn.PWT, cn.pjb_bc = kvb64, PWT, pjb_bc
        cn.dgp, cn.dgz, cn.dge, cn.dwcb, cn.masks = dgp, dgz, dge, dwcb, masks
        cn.ones_r, cn.ones_c, cn.XF, cn.vpad, cn.y_out = ones_r, ones_c, XF, vpad, y_out

        p0 = _phases(nc, 0, wp, pmm, pa, cn)
        p1 = _phases(nc, 1, wp, pmm, pa, cn)
        # x stages feed b0's K/V tiles just-in-time; batch-1 stages are spread
        # into the PE-rich Q/dwconv windows to keep ACT/DVE from oversubscribing
        prep_stage(0)
        prep_stage(1)
        p0["k_begin"]()
        for t in range(TT):
            prep_stage(t + 2)
            p0["k_tile"](t)
        for t in range(TT):
            if t < 6:
                prep_stage(10 + t)
            p0["v_tile"](t)
        p0["v_end"]()
        for g in range(KVH):
            p0["vt"](g)
        for g in range(KVH):
            p0["e1"](g)
        p0["q"](0, 2 * NK)
        p0["dwc"](None)
        p1["k_begin"]()
        for t in range(TT):
            p1["k_tile"](t)
        p0["norms"]()
        # b1's V tiles fill b0's einsum2 evacuation drain head-by-head
        fill0 = iter([(lambda t=t: p1["v_tile"](t)) for t in range(TT)])
        p0["e2"](0, fill0)
        fill1 = iter([p1["v_end"]]
                     + [(lambda g=g: p1["vt"](g)) for g in range(KVH)]
                     + [(lambda g=g: p1["e1"](g)) for g in range(KVH)])
        p0["e2"](1, fill1)
        p1["q"](0, 2)
        p0["proj"](0, 6)
        p1["q"](2, 2 * NK)
        p1["dwc"](None)
        p0["proj"](6, 8)
        p1["norms"]()
        p1["e2"](0)
        # b1's own proj tiles 0-3 (ready after e2 chunk 0) fill e2 chunk 1
        fill3 = iter(p1["proj_groups"](0, 4))
        p1["e2"](1, fill3)
        p1["proj"](4, 8)

    nc.compile()
    return nc


def _phases(nc, b, wp, pmm, pa, cn):
    st = _St()
    XF = cn.XF

    def emit_kv_half(t, vhalf, out_pk):
        t0 = b * N + 128 * t
        wf = cn.wkvfv if vhalf else cn.wkvfk
        w8 = cn.wkv8rv if vhalf else cn.wkv8rk
        for ci, cc in ((0, 0), (192, 192)):
            dst = out_pk[:, ci:ci + 192]
            for a in range(4):
                nc.tensor.matmul(dst, XF[:, 1, 2 * a:2 * a + 2, t0:t0 + 128],
                                 wf[:, 0, 2 * a:2 * a + 2, cc:cc + 192],
                                 start=(a == 0), stop=False, perf_mode=DR)
            nc.tensor.matmul(dst, XF[:, :, 8, t0:t0 + 128],
                             wf[:, :, 8, cc:cc + 192],
                             start=False, stop=False, perf_mode=DR)
            for k in range(8):
                nc.tensor.matmul(dst, XF[:, :, k, t0:t0 + 128],
                                 wf[:, :, k, cc:cc + 192],
                                 start=False, stop=False, perf_mode=DR)
            nc.tensor.matmul(dst, XF[:, :, 8, t0:t0 + 128],
                             w8[:, :, cc:cc + 192],
                             start=False, stop=False, perf_mode=DR)
            bc = 384 * vhalf + cc
            nc.tensor.matmul(dst, cn.ones_r[:], cn.kvb64[:, bc:bc + 192],
                             start=False, stop=True)

    def ph_k_begin():
        st.k3 = [wp.tile([128, 384], BF16, name=f"k3_{t}", tag=f"k3_{t}")
                 for t in range(TT)]
        st.vv = [wp.tile([128, 384], BF16, name=f"v_{t}", tag=f"v_{t}")
                 for t in range(TT)]
        st.acc2k = wp.tile([128, KVH * TT], F32, name="acc2k", tag="acc2k", bufs=2)
        st.uk2s = []

    def ph_k_tile(t):
        pk = pmm.tile([128, 512], F32, name="pk", tag="pmm", bufs=4)
        emit_kv_half(t, 0, pk)
        if t == 0:
            st.acc1kr = pmm.tile([1, 384], F32, name="acc1kr", tag="pdw",
                                 bufs=2)
        if t >= 2:
            # row-accumulate acc1k at a 2-tile lag so PE never waits on ACT
            nc.tensor.matmul(st.acc1kr[:], cn.ones_c[:], st.uk2s[t - 2][:],
                             start=(t == 2), stop=False)
        uk = wp.tile([128, 384], BF16, name="uk", tag="uk", bufs=2)
        nc.scalar.activation(uk[:], pk[:, 0:384], AF.Relu, scale=1.0 / SW)
        uk2 = wp.tile([128, 384], BF16, name="uk2", tag="uk2", bufs=3)
        st.uk2s.append(uk2)
        nc.vector.tensor_mul(uk2[:], uk[:], uk[:])
        nc.vector.tensor_mul(st.k3[t][:], uk2[:], uk[:])

    def ph_v_tile(t):
        # deferred from the K window (k3 persists): uk6 = k3^2 and the acc2k
        # group reductions run here where DVE has slack
        uk6 = wp.tile([128, 384], BF16, name="uk6", tag="uk6", bufs=2)
        nc.vector.tensor_mul(uk6[:], st.k3[t][:], st.k3[t][:])
        for g in range(KVH):
            nc.vector.tensor_reduce(st.acc2k[:, g * TT + t:g * TT + t + 1],
                                    uk6[:, 96 * g:96 * (g + 1)],
                                    axis=AX.X, op=ALU.add)
        pv = pmm.tile([128, 512], F32, name="pv", tag="pmm", bufs=4)
        if t < 2:
            # flush the lagged acc1k row-accumulation
            nc.tensor.matmul(st.acc1kr[:], cn.ones_c[:],
                             st.uk2s[TT - 2 + t][:],
                             start=False, stop=(t == 1))
        emit_kv_half(t, 1, pv)
        nc.scalar.activation(st.vv[t][:], pv[:, 0:384], AF.Copy, scale=1.0 / SW)
        nc.sync.dma_start(
            out=cn.vpad[b, 128 * t:128 * (t + 1), :, 0:96],
            in_=st.vv[t][:].rearrange("p (k d) -> p k d", k=KVH))

    def ph_v_end():
        # k-side acc1 group sums: free the pnorm psum row early
        st.kred1 = wp.tile([1, KVH], F32, name="kred1", tag="kred1", bufs=2)
        nc.vector.tensor_reduce(st.kred1[:],
                                st.acc1kr[:].rearrange("a (k d) -> a k d", k=KVH),
                                axis=AX.X, op=ALU.add)

    def ph_vt(g):
        if g == 0:
            st.vT8 = []
        vT = wp.tile([128, N], BF16, name="vTd", tag="vTd", bufs=2)
        nc.sync.dma_start(out=vT[:], in_=cn.vpad[b, :, g, :], transpose=True)
        v8 = wp.tile([96, N], FP8, name="v8", tag=f"v8_{g}")
        if g % 2 == 0:
            nc.scalar.activation(v8[:], vT[0:96, :], AF.Copy)
        else:
            nc.vector.tensor_copy(v8[:], vT[0:96, :])
        st.vT8.append(v8)

    def ph_e1(g):
        if g == 0:
            st.kvu = [wp.tile([96, 96], BF16, name=f"kvu_{gg}", tag=f"kvu_{gg}")
                      for gg in range(KVH)]
        pk_t = pa.tile([96, 96], F32, name="pkvt", tag="pa", bufs=2)
        for t in range(TT):
            nc.tensor.matmul(pk_t[:], st.k3[t][:, 96 * g:96 * (g + 1)],
                             st.vv[t][:, 96 * g:96 * (g + 1)],
                             start=(t == 0), stop=(t == TT - 1))
        nc.vector.tensor_copy(st.kvu[g][:], pk_t[:])

    def ph_q(lo, hi):
        if lo == 0:
            st.acc1q = wp.tile([128, NK * CH], F32, name="acc1q", tag="acc1q")
            st.acc2q = wp.tile([128, NK * CH], F32, name="acc2q", tag="acc2q")
            st.q3 = [wp.tile([128, N], BF16, name=f"q3_{j}", tag=f"q3_{j}")
                     for j in range(NK)]
        wqfj, wq8r = cn.wqfj, cn.wq8r
        for ci in range(lo, hi):
            c2, j = divmod(ci, NK)
            if True:
                pq = pmm.tile([128, 512], F32, name="pq", tag="pmm", bufs=4)
                for sub in range(2):
                    t0 = b * N + 512 * c2 + 256 * sub
                    dst = pq[:, 256 * sub:256 * (sub + 1)]
                    for a in range(4):
                        nc.tensor.matmul(dst, wqfj[:, j, 0, 2 * a:2 * a + 2, :],
                                         XF[:, 1, 2 * a:2 * a + 2, t0:t0 + 256],
                                         start=(a == 0), stop=False, perf_mode=DR)
                    nc.tensor.matmul(dst, wqfj[:, j, :, 8, :],
                                     XF[:, :, 8, t0:t0 + 256],
                                     start=False, stop=False, perf_mode=DR)
                    for k in range(8):
                        nc.tensor.matmul(dst, wqfj[:, j, :, k, :],
                                         XF[:, :, k, t0:t0 + 256],
                                         start=False, stop=False, perf_mode=DR)
                    nc.tensor.matmul(dst, wq8r[:, :, 128 * j:128 * (j + 1)],
                                     XF[:, :, 8, t0:t0 + 256],
                                     start=False, stop=(sub == 1), perf_mode=DR)
                u = wp.tile([128, 512], BF16, name="u", tag="u", bufs=2)
                nc.scalar.activation(u[:], pq[:], AF.Relu, scale=1.0 / SW,
                                     bias=cn.wqb[:, j:j + 1])
                u2 = wp.tile([128, 512], BF16, name="u2", tag="u2", bufs=2)
                col = j * CH + c2
                nc.scalar.activation(u2[:], u[:], AF.Square,
                                     accum_out=st.acc1q[:, col:col + 1])
                q3s = st.q3[j][:, 512 * c2:512 * (c2 + 1)]
                nc.vector.tensor_mul(q3s, u2[:], u[:])
                u6 = wp.tile([128, 512], BF16, name="u6", tag="u6", bufs=2)
                nc.vector.tensor_mul(u6[:], q3s, q3s)
                nc.vector.tensor_reduce(st.acc2q[:, col:col + 1], u6[:],
                                        axis=AX.X, op=ALU.add)

    def ph_dwc(hook):
        st.vdwc = [wp.tile([96, N], BF16, name=f"vdwc_{g}", tag=f"vdwc_{g}")
                   for g in range(KVH)]
        for g in range(KVH):
            if hook is not None:
                hook(g)
            v3 = st.vT8[g][:].rearrange("p (y x) -> p y x", y=32)
            for hf in range(2):
                pd = pmm.tile([96, 512], F32, name="pd", tag="pdw", bufs=2)
                p3 = pd[:].rearrange("p (y x) -> p y x", y=16)
                mms = []
                for dxi, dx in enumerate((-1, 0, 1)):
                    x0, x1 = max(0, -dx), 32 - max(0, dx)
                    # dy=0 tap paired with a zero-weight slot -> DoubleRow rate
                    base = v3[0:96, 16 * hf:16 * hf + 16, x0 + dx:x1 + dx]
                    mms.append((cn.dgz[:, :, g, dxi, :], _ins_dim(base, 0, 2),
                                p3[:, 0:16, x0:x1], DR))
                ya0 = max(1, 16 * hf)
                ya1 = min(31, 16 * hf + 16)
                for dxi, dx in enumerate((-1, 0, 1)):
                    x0, x1 = max(0, -dx), 32 - max(0, dx)
                    base = v3[0:96, ya0 - 1:ya1 - 1, x0 + dx:x1 + dx]
                    rhs = _ins_dim(base, 64, 2)
                    mms.append((cn.dgp[:, :, g, dxi, :], rhs,
                                p3[:, ya0 - 16 * hf:ya1 - 16 * hf, x0:x1], DR))
                for dxi, dx in enumerate((-1, 0, 1)):
                    x0, x1 = max(0, -dx), 32 - max(0, dx)
                    if hf == 0:  # y=0, tap dy=+1
                        mms.append((cn.dge[:, g, 6 + dxi, :],
                                    v3[0:96, 1:2, x0 + dx:x1 + dx],
                                    p3[:, 0:1, x0:x1], None))
                    else:        # y=31, tap dy=-1
                        mms.append((cn.dge[:, g, dxi, :],
                                    v3[0:96, 30:31, x0 + dx:x1 + dx],
                                    p3[:, 15:16, x0:x1], None))
                for mi, (lhsT, rhs, out, pm) in enumerate(mms):
                    nc.tensor.matmul(out, lhsT, rhs, start=(mi == 0),
                                     stop=(mi == len(mms) - 1), perf_mode=pm)
                nc.scalar.activation(st.vdwc[g][:, 512 * hf:512 * (hf + 1)],
                                     pd[:], AF.Identity, scale=1.0 / SW,
                                     bias=cn.dwcb[:, g:g + 1])

    def ph_norms():
        sq_rows = []
        for ai, acc in enumerate((st.acc1q, st.acc2q)):
            accs = wp.tile([128, NK], F32, name="accs", tag="accs", bufs=2)
            av = acc[:, 0:NK * CH].rearrange("p (j c) -> p j c", c=CH)
            nc.vector.tensor_add(accs[:], av[:, :, 0], av[:, :, 1])
            accsb = wp.tile([128, NK], BF16, name="accsb", tag="accsb", bufs=2)
            nc.vector.tensor_copy(accsb[:], accs[:])
            psn = pa.tile([1, H], F32, name="psn", tag="pa", bufs=2)
            for j in range(NK):
                nc.tensor.matmul(psn[:], accsb[:, j:j + 1], cn.masks[:, j, :],
                                 start=(j == 0), stop=(j == NK - 1))
            srow = wp.tile([1, H], F32, name="srow", tag="srow", bufs=4)
            nc.vector.tensor_copy(srow[:], psn[:])
            sq_rows.append(srow)
        acc2kb = wp.tile([128, KVH * TT], BF16, name="acc2kb", tag="acc2kb",
                         bufs=2)
        nc.vector.tensor_copy(acc2kb[:], st.acc2k[:])
        psk = pa.tile([1, KVH * TT], F32, name="psk", tag="pa", bufs=2)
        nc.tensor.matmul(psk[:], cn.ones_c[:], acc2kb[:], start=True, stop=True)
        krow = wp.tile([1, KVH * TT], F32, name="krow", tag="krow", bufs=2)
        nc.vector.tensor_copy(krow[:], psk[:])
        kred2 = wp.tile([1, KVH], F32, name="kred2", tag="kred2", bufs=2)
        nc.vector.tensor_reduce(kred2[:],
                                krow[:].rearrange("a (k t) -> a k t", k=KVH),
                                axis=AX.X, op=ALU.add)
        sk_rows = [st.kred1, kred2]

        def _f_row(s1, s2, width, tagp):
            se = wp.tile([1, width], F32, name="se", tag=f"se{tagp}", bufs=2)
            nc.vector.tensor_scalar_add(se[:], s2[:], 1e-30)
            rc = wp.tile([1, width], F32, name="rc", tag=f"rc{tagp}", bufs=2)
            nc.vector.reciprocal(rc[:], se[:])
            rt = wp.tile([1, width], F32, name="rt", tag=f"rt{tagp}", bufs=2)
            nc.vector.tensor_mul(rt[:], s1[:], rc[:])
            fr = wp.tile([1, width], F32, name="fr", tag=f"fr{tagp}", bufs=2)
            nc.scalar.activation(fr[:], rt[:], AF.Sqrt)
            return fr

        fq = _f_row(sq_rows[0], sq_rows[1], H, "q")
        fk = _f_row(sk_rows[0], sk_rows[1], KVH, "k")
        fk12 = wp.tile([1, H], F32, name="fk12", tag="fk12", bufs=2)
        for g in range(3):
            nc.vector.tensor_copy(fk12[:, 4 * g:4 * (g + 1)], fk[:])
        grow = wp.tile([1, H], F32, name="grow", tag="grow", bufs=2)
        nc.vector.tensor_mul(grow[:], fq[:], fk12[:])
        gb = wp.tile([96, H], F32, name="gb", tag="gb", bufs=2)
        nc.gpsimd.partition_broadcast(gb[:], grow[:], channels=96)
        st.kvp = [wp.tile([96, 96], BF16, name=f"kvp_{h}", tag=f"kvp_{h}")
                  for h in range(H)]
        for h in range(H):
            nc.vector.tensor_scalar_mul(st.kvp[h][:], st.kvu[h % KVH][:],
                                        gb[:, h:h + 1])

    def ph_e2(c2, filler=None):
        if c2 == 0:
            st.OT = [wp.tile([128, N], BF16, name=f"OT_{j}", tag=f"OT_{j}")
                     for j in range(NK)]
        for h in range(H):
            if filler is not None:
                fn = next(filler, None)
                if fn is not None:
                    fn()
            pieces = _head_pieces(h)
            if len(pieces) == 1:
                j0, r00, _, _ = pieces[0]
                rhs = st.q3[j0][r00:r00 + 96, 512 * c2:512 * (c2 + 1)]
            else:
                qh = wp.tile([96, 512], BF16, name="qh", tag="qh", bufs=4)
                for pi, (j, r0, rr, cnt) in enumerate(pieces):
                    src_ap = st.q3[j][r0:r0 + cnt, 512 * c2:512 * (c2 + 1)]
                    if (h + pi) % 2 == 0:
                        nc.vector.tensor_copy(qh[rr:rr + cnt, :], src_ap)
                    else:
                        nc.scalar.copy(qh[rr:rr + cnt, :], src_ap)
                rhs = qh[:]
            pa_t = pa.tile([96, 512], F32, name="pat", tag="pa", bufs=2)
            nc.tensor.matmul(pa_t[:], st.kvp[h][:], rhs, start=True,
                             stop=True)
            if len(pieces) == 1:
                j0, r00, _, _ = pieces[0]
                nc.vector.tensor_tensor(
                    out=st.OT[j0][r00:r00 + 96, 512 * c2:512 * (c2 + 1)],
                    in0=pa_t[:],
                    in1=st.vdwc[h % KVH][:, 512 * c2:512 * (c2 + 1)],
                    op=ALU.add)
            else:
                pac = wp.tile([96, 512], BF16, name="pac", tag="pac", bufs=4)
                nc.scalar.copy(pac[:], pa_t[:])
                for (j, r0, rr, cnt) in pieces:
                    nc.vector.tensor_tensor(
                        out=st.OT[j][r0:r0 + cnt, 512 * c2:512 * (c2 + 1)],
                        in0=pac[rr:rr + cnt, :],
                        in1=st.vdwc[h % KVH][rr:rr + cnt,
                                             512 * c2:512 * (c2 + 1)],
                        op=ALU.add)

    def _proj_group(t, oc):
        py = pmm.tile([128, 384], F32, name="py", tag="pdw", bufs=2)
        for j in range(NK):
            nc.tensor.matmul(py[:], st.OT[j][:, 128 * t:128 * (t + 1)],
                             cn.PWT[j][:, 384 * oc:384 * (oc + 1)],
                             start=(j == 0), stop=(j == NK - 1))
        ysb = wp.tile([128, 384], F32, name="ysb", tag="ysb", bufs=3)
        # bias rides the psum evacuation (pjb_bc pre-broadcast on host)
        nc.vector.tensor_tensor(out=ysb[:], in0=py[:],
                                in1=cn.pjb_bc[:, 384 * oc:384 * (oc + 1)],
                                op=ALU.add)
        t0 = b * N + 128 * t
        nc.sync.dma_start(out=cn.y_out[t0:t0 + 128, 384 * oc:384 * (oc + 1)],
                          in_=ysb[:])

    def ph_proj_groups(ta, tb):
        return [(lambda t=t, oc=oc: _proj_group(t, oc))
                for t in range(ta, tb) for oc in range(3)]

    def ph_proj(ta, tb):
        for fn in ph_proj_groups(ta, tb):
            fn()

    return dict(k_begin=ph_k_begin, k_tile=ph_k_tile, v_tile=ph_v_tile,
                v_end=ph_v_end, vt=ph_vt, e1=ph_e1, q=ph_q,
                dwc=ph_dwc, norms=ph_norms, e2=ph_e2, proj=ph_proj,
                proj_groups=ph_proj_groups)


_NC_CACHE = None


def _get_nc():
    global _NC_CACHE
    if _NC_CACHE is None:
        _NC_CACHE = _build_kernel()
    return _NC_CACHE


def _hi_lo(a):
    hi = a.astype(_F8)
    lo = (a - hi.astype(np.float32)).astype(_F8)
    return hi, lo


def _host_consts(wq_w, wq_b, wkv_w, wkv_b, dwc_w, dwc_b, proj_w, proj_b):
    wqT = np.ascontiguousarray(np.asarray(wq_w, np.float32).T) * SW      # [in, out]
    wkvT = np.ascontiguousarray(np.asarray(wkv_w, np.float32).T) * SW    # [in, 768]
    qhi, qlo = _hi_lo(wqT)
    khi, klo = _hi_lo(wkvT)

    # wqfj: [128, j, slot(hi,lo), k, 128]
    wqfj = np.zeros((128, NK, 2, NK, 128), _F8)
    for k in range(NK):
        for j in range(NK):
            wqfj[:, j, 0, k, :] = qhi[128 * k:128 * (k + 1), 128 * j:128 * (j + 1)]
            wqfj[:, j, 1, k, :] = qlo[128 * k:128 * (k + 1), 128 * j:128 * (j + 1)]
    wq8r = np.zeros((128, 2, DIM), _F8)
    wq8r[:, 0, :] = qlo[128 * 8:, :]
    wq8r[:, 1, :] = qhi[128 * 8:, :]

    wkvf = np.zeros((128, 2, NK, 768), _F8)
    for k in range(NK):
        wkvf[:, 0, k, :] = khi[128 * k:128 * (k + 1), :]
        wkvf[:, 1, k, :] = klo[128 * k:128 * (k + 1), :]
    wkv8r = np.zeros((128, 2, 768), _F8)
    wkv8r[:, 0, :] = klo[128 * 8:, :]
    wkv8r[:, 1, :] = khi[128 * 8:, :]
    wkvfk = np.ascontiguousarray(wkvf[:, :, :, 0:384])
    wkvfv = np.ascontiguousarray(wkvf[:, :, :, 384:768])
    wkv8rk = np.ascontiguousarray(wkv8r[:, :, 0:384])
    wkv8rv = np.ascontiguousarray(wkv8r[:, :, 384:768])

    pwT = np.ascontiguousarray(np.asarray(proj_w, np.float32).T).astype(_BF)
    wqb = np.ascontiguousarray(np.asarray(wq_b, np.float32).reshape(NK, 128).T)
    kvb64 = (np.asarray(wkv_b, np.float32).reshape(1, 768) * SW).astype(_BF)
    pjb_bc = np.broadcast_to(np.asarray(proj_b, np.float32).reshape(1, DIM),
                             (128, DIM)).astype(_BF)

    dw = np.asarray(dwc_w, np.float32).reshape(KVH, 96, 9) * SW  # [g, d, tap]
    dgp = np.zeros((96, 2, KVH, 3, 96), np.float32)
    dgz = np.zeros((96, 2, KVH, 3, 96), np.float32)
    dge = np.zeros((96, KVH, 9, 96), np.float32)
    for d in range(96):
        for dxi in range(3):
            dgp[d, 0, :, dxi, d] = dw[:, d, 0 + dxi]       # dy=-1 taps 0,1,2
            dgp[d, 1, :, dxi, d] = dw[:, d, 6 + dxi]       # dy=+1 taps 6,7,8
            dgz[d, 0, :, dxi, d] = dw[:, d, 3 + dxi]       # dy=0 taps, slot1=0
        for ti in range(9):
            dge[d, :, ti, d] = dw[:, d, ti]
    dgp = dgp.astype(_F8)
    dgz = dgz.astype(_F8)
    dge = dge.astype(_F8)
    dwcb = np.ascontiguousarray(np.asarray(dwc_b, np.float32).reshape(KVH, 96).T)

    mk = np.zeros((128, NK, H), np.float32)
    for j in range(NK):
        for p in range(128):
            f = 128 * j + p
            mk[p, j, f // 96] = 1.0
    masks = mk.astype(_BF)
    eye = np.eye(128, dtype=np.float32).astype(_BF)
    return dict(wqfj=wqfj, wq8r=wq8r, wkvfk=wkvfk, wkvfv=wkvfv, wkv8rk=wkv8rk,
                wkv8rv=wkv8rv, pwT=pwT, wqb=wqb, kvb64=kvb64, pjb_bc=pjb_bc,
                dgp=dgp, dgz=dgz, dge=dge, dwcb=dwcb, masks=masks, eye=eye)


def kernel(x, wq_w, wq_b, wkv_w, wkv_b, dwc_w, dwc_b, proj_w, proj_b,
           _want_results=False, **_unused):
    nc = _get_nc()
    consts = _host_consts(wq_w, wq_b, wkv_w, wkv_b, dwc_w, dwc_b, proj_w, proj_b)
    x = np.asarray(x, np.float32)
    in_maps = []
    for c in range(NCORES):
        m = dict(consts)
        m["x"] = np.ascontiguousarray(x[BL * c:BL * (c + 1)].reshape(T, DIM))
        in_maps.append(m)
    res = bass_utils.run_bass_kernel_spmd(nc, in_maps, core_ids=list(range(NCORES)))
    y = np.stack([res.results[c]["y"].reshape(BL, N, DIM) for c in range(NCORES)])
    y = y.reshape(B, N, DIM)
    if _want_results:
        return y, res
    return y


# revision 62
# speedup vs baseline: 1.0007x; 1.0007x over previous
"""Trainium2 Bass kernel for DiT focused-linear-attention block (nn_DiT_9259949490457).

Data-parallel over batch: 16 batches -> 8 NeuronCores, 2 batches/core, no collectives.

q and kv GEMMs run in fp8-e4m3 hi/lo split-precision using DoubleRow perf mode
(2 slice-products per 0.5-cycle/row instruction -> 14 DR units vs 18 bf16 units per
K=1152 contraction, with better-than-bf16 accuracy). The hi/lo fp8 cast of x is fused
into the transpose-psum evacuation (ACT writes hi, DVE subtract writes lo). The
depthwise 3x3 conv branch pairs (dy=-1,dy=+1) taps into DoubleRow fp8 matmuls via
custom overlapping access patterns.

The two local batches are phase-interleaved so PE always has matmul work while the
other batch's norm chain / einsum2-evacuation drains on ACT/DVE: x-prep stages feed
b0's K/V tiles just-in-time; b1's K fills b0's norm chain; b1's V/vT/e1 tiles are
emitted head-by-head as fillers inside b0's einsum2; b1's own proj tiles 0-3 fill
its einsum2 chunk 1; PSUM runs 4-deep on the GEMM tag with the norm row sharing
the dwconv/proj bank tag.

Self-contained: hardcodes all shapes; host numpy pre-packs fp8 hi/lo weights
(scaled by 64; 1/64 folded into psum-evacuation activations).
"""

import numpy as np
import ml_dtypes
import bass_rust

import concourse.bacc as bacc
import concourse.mybir as mybir
import concourse.tile as tile
from concourse import bass_utils

F32 = mybir.dt.float32
BF16 = mybir.dt.bfloat16
FP8 = mybir.dt.float8e4
ALU = mybir.AluOpType
AF = mybir.ActivationFunctionType
AX = mybir.AxisListType
DR = mybir.MatmulPerfMode.DoubleRow

NCORES = 8
B, N, DIM = 16, 1024, 1152
H, KVH, HD = 12, 4, 96
BL = B // NCORES          # 2 local batches
T = BL * N                # 2048 local tokens
NK = DIM // 128           # 9 feature K-slices
TT = N // 128             # 8 token tiles per batch
CH = N // 512             # 2 free-dim chunks of 512 per batch
SW = 64.0                 # fp8 weight pre-scale (power of two)

_BF = ml_dtypes.bfloat16
_F8 = ml_dtypes.float8_e4m3fn


def _spanp(b):
    if b % 128 == 0:
        return 128
    if b % 64 == 0:
        return 64
    return 32


def _head_pieces(h):
    out = []
    rr = 0
    while rr < 96:
        gr = 96 * h + rr
        j, r0 = divmod(gr, 128)
        cnt = min(96 - rr, 128 - r0, _spanp(r0), _spanp(rr))
        out.append((j, r0, rr, cnt))
        rr += cnt
    return out


def _ins_dim(ap, stride, count):
    """Insert a free dim [stride, count] right after the partition dim."""
    dims = [list(d) for d in ap.ap]
    new = [dims[0], [stride, count]] + dims[1:]
    return bass_rust.AP(ap.tensor, ap.offset, new)


class _St:
    pass


def _build_kernel():
    nc = bacc.Bacc("TRN2", target_bir_lowering=False, debug=False,
                   enable_asserts=True, num_devices=NCORES)
    x_in = nc.dram_tensor("x", [T, DIM], F32, kind="ExternalInput").ap()
    wqfj_in = nc.dram_tensor("wqfj", [128, NK, 2, NK, 128], FP8, kind="ExternalInput").ap()
    wq8r_in = nc.dram_tensor("wq8r", [128, 2, DIM], FP8, kind="ExternalInput").ap()
    wkvfk_in = nc.dram_tensor("wkvfk", [128, 2, NK, 384], FP8, kind="ExternalInput").ap()
    wkvfv_in = nc.dram_tensor("wkvfv", [128, 2, NK, 384], FP8, kind="ExternalInput").ap()
    wkv8rk_in = nc.dram_tensor("wkv8rk", [128, 2, 384], FP8, kind="ExternalInput").ap()
    wkv8rv_in = nc.dram_tensor("wkv8rv", [128, 2, 384], FP8, kind="ExternalInput").ap()
    pwT_in = nc.dram_tensor("pwT", [DIM, DIM], BF16, kind="ExternalInput").ap()
    wqb_in = nc.dram_tensor("wqb", [128, NK], F32, kind="ExternalInput").ap()
    kvb64_in = nc.dram_tensor("kvb64", [1, 768], BF16, kind="ExternalInput").ap()
    pjb_bc_in = nc.dram_tensor("pjb_bc", [128, DIM], BF16, kind="ExternalInput").ap()
    dgp_in = nc.dram_tensor("dgp", [96, 2, KVH, 3, 96], FP8, kind="ExternalInput").ap()
    dgz_in = nc.dram_tensor("dgz", [96, 2, KVH, 3, 96], FP8, kind="ExternalInput").ap()
    dge_in = nc.dram_tensor("dge", [96, KVH, 9, 96], FP8, kind="ExternalInput").ap()
    dwcb_in = nc.dram_tensor("dwcb", [96, KVH], F32, kind="ExternalInput").ap()
    masks_in = nc.dram_tensor("masks", [128, NK, H], BF16, kind="ExternalInput").ap()
    eye_in = nc.dram_tensor("eye", [128, 128], BF16, kind="ExternalInput").ap()
    y_out = nc.dram_tensor("y", [T, DIM], F32, kind="ExternalOutput").ap()

    from contextlib import ExitStack
    with tile.TileContext(nc) as tc, ExitStack() as stack:
        cpool = stack.enter_context(tc.tile_pool(name="const", bufs=1))
        dpool = stack.enter_context(tc.tile_pool(name="dram", bufs=1, space="DRAM"))

        # ---- consts / weights ----
        eye = cpool.tile([128, 128], BF16, name="eye")
        wkvfk = cpool.tile([128, 2, NK, 384], FP8, name="wkvfk")
        wkvfv = cpool.tile([128, 2, NK, 384], FP8, name="wkvfv")
        wkv8rk = cpool.tile([128, 2, 384], FP8, name="wkv8rk")
        wkv8rv = cpool.tile([128, 2, 384], FP8, name="wkv8rv")
        kvb64 = cpool.tile([1, 768], BF16, name="kvb64")
        wqfj = cpool.tile([128, NK, 2, NK, 128], FP8, name="wqfj")
        wq8r = cpool.tile([128, 2, DIM], FP8, name="wq8r")
        wqb = cpool.tile([128, NK], F32, name="wqb")
        PWT = [cpool.tile([128, DIM], BF16, name=f"PWT{k}") for k in range(NK)]
        pjb_bc = cpool.tile([128, DIM], BF16, name="pjb_bc")
        dgp = cpool.tile([96, 2, KVH, 3, 96], FP8, name="dgp")
        dgz = cpool.tile([96, 2, KVH, 3, 96], FP8, name="dgz")
        dge = cpool.tile([96, KVH, 9, 96], FP8, name="dge")
        dwcb = cpool.tile([96, KVH], F32, name="dwcb")
        masks = cpool.tile([128, NK, H], BF16, name="masks")
        ones_r = cpool.tile([1, 128], BF16, name="ones_r")
        ones_c = cpool.tile([128, 1], BF16, name="ones_c")
        sqwarm = cpool.tile([1, 8], F32, name="sqwarm")

        vpad = dpool.tile([BL, N, KVH, 128], BF16, name="vpad")

        # ---- pools ----
        xpool = stack.enter_context(tc.tile_pool(name="xf", bufs=1))
        XF = xpool.tile([128, 2, NK, T], FP8, name="XF")
        wp = stack.enter_context(tc.tile_pool(name="work", bufs=1))
        pmm = stack.enter_context(tc.tile_pool(name="pmm", bufs=1, space="PSUM"))
        pa = stack.enter_context(tc.tile_pool(name="pa", bufs=1, space="PSUM"))

        # ---- prologue machinery: load x, transpose on PE, evacuate as fp8
        # hi/lo into XF. Stages are interleaved with batch-0 K/V tiles so the
        # ACT/DVE evacuation queue never runs ahead of the GEMM consumers.
        prep = stack.enter_context(tc.tile_pool(name="prep", bufs=3))

        def prep_stage(i):
            stage = prep.tile([128, DIM], BF16, name="stage", tag="stage")
            nc.gpsimd.dma_start(out=stage[:],
                                in_=x_in[128 * i:128 * (i + 1), :])
            if i == 0:
                nc.sync.dma_start(out=eye[:], in_=eye_in[:])
                nc.vector.memset(ones_r[:], 1.0)
                nc.vector.memset(ones_c[:], 1.0)
                nc.vector.memset(sqwarm[:], 1.0)
                # warm the Sqrt activation table off the critical path
                nc.scalar.activation(sqwarm[:], sqwarm[:], AF.Sqrt)
            elif i == 1:
                nc.sync.dma_start(out=wkvfk[:], in_=wkvfk_in[:])
                nc.sync.dma_start(out=wkv8rk[:], in_=wkv8rk_in[:])
                nc.sync.dma_start(out=kvb64[:], in_=kvb64_in[:])
            elif i == 3:
                nc.sync.dma_start(out=wkvfv[:], in_=wkvfv_in[:])
                nc.sync.dma_start(out=wkv8rv[:], in_=wkv8rv_in[:])
            elif i == 5:
                for j in range(NK):
                    nc.sync.dma_start(out=wqfj[:, j], in_=wqfj_in[:, j])
                nc.sync.dma_start(out=wq8r[:], in_=wq8r_in[:])
                nc.sync.dma_start(out=wqb[:], in_=wqb_in[:])
            elif i == 9:
                for k in range(NK):
                    nc.sync.dma_start(out=PWT[k][:],
                                      in_=pwT_in[128 * k:128 * (k + 1), :])
                nc.sync.dma_start(out=pjb_bc[:], in_=pjb_bc_in[:])
            elif i == 11:
                nc.sync.dma_start(out=dgp[:], in_=dgp_in[:])
                nc.sync.dma_start(out=dgz[:], in_=dgz_in[:])
                nc.sync.dma_start(out=dge[:], in_=dge_in[:])
                nc.sync.dma_start(out=dwcb[:], in_=dwcb_in[:])
                nc.sync.dma_start(out=masks[:], in_=masks_in[:])
            pt_a = pa.tile([128, 1024], BF16, name="pt_a", tag="pa", bufs=2)
            for k in range(8):
                nc.tensor.transpose(pt_a[:, 128 * k:128 * (k + 1)],
                                    stage[:, 128 * k:128 * (k + 1)], eye[:])
            pt_b = pa.tile([128, 128], BF16, name="pt_b", tag="pa", bufs=2)
            nc.tensor.transpose(pt_b[:], stage[:, 1024:1152], eye[:])
            cs = slice(128 * i, 128 * (i + 1))
            pt3 = pt_a[:].rearrange("p (k c) -> p k c", c=128)
            # hi casts (ACT), slices 0-7 slot1, slice 8 slot0
            nc.scalar.activation(XF[:, 1, 0:8, cs], pt3, AF.Copy)
            nc.scalar.activation(XF[:, 0, 8, cs], pt_b[:], AF.Copy)
            # lo = psum - hi (DVE), slices 0-7 slot0, slice 8 slot1
            nc.vector.tensor_tensor(out=XF[:, 0, 0:8, cs], in0=pt3,
                                    in1=XF[:, 1, 0:8, cs], op=ALU.subtract)
            nc.vector.tensor_tensor(out=XF[:, 1, 8, cs], in0=pt_b[:],
                                    in1=XF[:, 0, 8, cs], op=ALU.subtract)

        cn = _St()
        cn.wqfj, cn.wq8r, cn.wqb = wqfj, wq8r, wqb
        cn.wkvfk, cn.wkvfv, cn.wkv8rk, cn.wkv8rv = wkvfk, wkvfv, wkv8rk, wkv8rv
        cn.kvb64, cn.PWT, cn.pjb_bc = kvb64, PWT, pjb_bc
        cn.dgp, cn.dgz, cn.dge, cn.dwcb, cn.masks = dgp, dgz, dge, dwcb, masks
        cn.ones_r, cn.ones_c, cn.XF, cn.vpad, cn.y_out = ones_r, ones_c, XF, vpad, y_out

        p0 = _phases(nc, 0, wp, pmm, pa, cn)
        p1 = _phases(nc, 1, wp, pmm, pa, cn)
        # x stages feed b0's K/V tiles just-in-time; batch-1 stages are spread
        # into the PE-rich Q/dwconv windows to keep ACT/DVE from oversubscribing
        prep_stage(0)
        prep_stage(1)
        p0["k_begin"]()
        for t in range(TT):
            prep_stage(t + 2)
            p0["k_tile"](t)
        for t in range(TT):
            if t < 6:
                prep_stage(10 + t)
            p0["v_tile"](t)
        p0["v_end"]()
        for g in range(KVH):
            p0["vt"](g)
        for g in range(KVH):
            p0["e1"](g)
        p0["q"](0, 2 * NK)
        p0["dwc"](None)
        p1["k_begin"]()
        for t in range(TT):
            p1["k_tile"](t)
        p0["norms"]()
        # b1's V tiles fill b0's einsum2 evacuation drain head-by-head
        fill0 = iter([(lambda t=t: p1["v_tile"](t)) for t in range(TT)])
        p0["e2"](0, fill0)
        fill1 = iter([p1["v_end"]]
                     + [(lambda g=g: p1["vt"](g)) for g in range(KVH)]
                     + [(lambda g=g: p1["e1"](g)) for g in range(KVH)])
        p0["e2"](1, fill1)
        p1["q"](0, 2)
        p0["proj"](0, 6)
        p1["q"](2, 2 * NK)
        p1["dwc"](None)
        p0["proj"](6, 8)
        p1["norms"]()
        p1["e2"](0)
        # b1's own proj tiles 0-3 (ready after e2 chunk 0) fill e2 chunk 1
        fill3 = iter(p1["proj_groups"](0, 4))
        p1["e2"](1, fill3)
        p1["proj"](4, 8)

    nc.compile()
    return nc


def _phases(nc, b, wp, pmm, pa, cn):
    st = _St()
    XF = cn.XF

    def emit_kv_half(t, vhalf, out_pk):
        t0 = b * N + 128 * t
        wf = cn.wkvfv if vhalf else cn.wkvfk
        w8 = cn.wkv8rv if vhalf else cn.wkv8rk
        for ci, cc in ((0, 0), (192, 192)):
            dst = out_pk[:, ci:ci + 192]
            for a in range(4):
                nc.tensor.matmul(dst, XF[:, 1, 2 * a:2 * a + 2, t0:t0 + 128],
                                 wf[:, 0, 2 * a:2 * a + 2, cc:cc + 192],
                                 start=(a == 0), stop=False, perf_mode=DR)
            nc.tensor.matmul(dst, XF[:, :, 8, t0:t0 + 128],
                             wf[:, :, 8, cc:cc + 192],
                             start=False, stop=False, perf_mode=DR)
            for k in range(8):
                nc.tensor.matmul(dst, XF[:, :, k, t0:t0 + 128],
                                 wf[:, :, k, cc:cc + 192],
                                 start=False, stop=False, perf_mode=DR)
            nc.tensor.matmul(dst, XF[:, :, 8, t0:t0 + 128],
                             w8[:, :, cc:cc + 192],
                             start=False, stop=False, perf_mode=DR)
            bc = 384 * vhalf + cc
            nc.tensor.matmul(dst, cn.ones_r[:], cn.kvb64[:, bc:bc + 192],
                             start=False, stop=True)

    def ph_k_begin():
        st.k3 = [wp.tile([128, 384], BF16, name=f"k3_{t}", tag=f"k3_{t}")
                 for t in range(TT)]
        st.vv = [wp.tile([128, 384], BF16, name=f"v_{t}", tag=f"v_{t}")
                 for t in range(TT)]
        st.acc2k = wp.tile([128, KVH * TT], F32, name="acc2k", tag="acc2k", bufs=2)
        st.uk2s = []

    def ph_k_tile(t):
        pk = pmm.tile([128, 512], F32, name="pk", tag="pmm", bufs=4)
        emit_kv_half(t, 0, pk)
        if t == 0:
            st.acc1kr = pmm.tile([1, 384], F32, name="acc1kr", tag="pdw",
                                 bufs=2)
        if t >= 2:
            # row-accumulate acc1k at a 2-tile lag so PE never waits on ACT
            nc.tensor.matmul(st.acc1kr[:], cn.ones_c[:], st.uk2s[t - 2][:],
                             start=(t == 2), stop=False)
        uk = wp.tile([128, 384], BF16, name="uk", tag="uk", bufs=2)
        nc.scalar.activation(uk[:], pk[:, 0:384], AF.Relu, scale=1.0 / SW)
        uk2 = wp.tile([128, 384], BF16, name="uk2", tag="uk2", bufs=3)
        st.uk2s.append(uk2)
        nc.scalar.activation(uk2[:], uk[:], AF.Square)
        nc.vector.tensor_mul(st.k3[t][:], uk2[:], uk[:])
        uk6 = wp.tile([128, 384], BF16, name="uk6", tag="uk6", bufs=2)
        nc.vector.tensor_mul(uk6[:], st.k3[t][:], st.k3[t][:])
        for g in range(KVH):
            nc.vector.tensor_reduce(st.acc2k[:, g * TT + t:g * TT + t + 1],
                                    uk6[:, 96 * g:96 * (g + 1)],
                                    axis=AX.X, op=ALU.add)

    def ph_v_tile(t):
        pv = pmm.tile([128, 512], F32, name="pv", tag="pmm", bufs=4)
        if t < 2:
            # flush the lagged acc1k row-accumulation
            nc.tensor.matmul(st.acc1kr[:], cn.ones_c[:],
                             st.uk2s[TT - 2 + t][:],
                             start=False, stop=(t == 1))
        emit_kv_half(t, 1, pv)
        nc.scalar.activation(st.vv[t][:], pv[:, 0:384], AF.Copy, scale=1.0 / SW)
        nc.sync.dma_start(
            out=cn.vpad[b, 128 * t:128 * (t + 1), :, 0:96],
            in_=st.vv[t][:].rearrange("p (k d) -> p k d", k=KVH))

    def ph_v_end():
        # k-side acc1 group sums: free the pnorm psum row early
        st.kred1 = wp.tile([1, KVH], F32, name="kred1", tag="kred1", bufs=2)
        nc.vector.tensor_reduce(st.kred1[:],
                                st.acc1kr[:].rearrange("a (k d) -> a k d", k=KVH),
                                axis=AX.X, op=ALU.add)

    def ph_vt(g):
        if g == 0:
            st.vT8 = []
        vT = wp.tile([128, N], BF16, name="vTd", tag="vTd", bufs=2)
        nc.sync.dma_start(out=vT[:], in_=cn.vpad[b, :, g, :], transpose=True)
        v8 = wp.tile([96, N], FP8, name="v8", tag=f"v8_{g}")
        if g % 2 == 0:
            nc.scalar.activation(v8[:], vT[0:96, :], AF.Copy)
        else:
            nc.vector.tensor_copy(v8[:], vT[0:96, :])
        st.vT8.append(v8)

    def ph_e1(g):
        if g == 0:
            st.kvu = [wp.tile([96, 96], BF16, name=f"kvu_{gg}", tag=f"kvu_{gg}")
                      for gg in range(KVH)]
        pk_t = pa.tile([96, 96], F32, name="pkvt", tag="pa", bufs=2)
        for t in range(TT):
            nc.tensor.matmul(pk_t[:], st.k3[t][:, 96 * g:96 * (g + 1)],
                             st.vv[t][:, 96 * g:96 * (g + 1)],
                             start=(t == 0), stop=(t == TT - 1))
        nc.vector.tensor_copy(st.kvu[g][:], pk_t[:])

    def ph_q(lo, hi):
        if lo == 0:
            st.acc1q = wp.tile([128, NK * CH], F32, name="acc1q", tag="acc1q")
            st.acc2q = wp.tile([128, NK * CH], F32, name="acc2q", tag="acc2q")
            st.q3 = [wp.tile([128, N], BF16, name=f"q3_{j}", tag=f"q3_{j}")
                     for j in range(NK)]
        wqfj, wq8r = cn.wqfj, cn.wq8r
        for ci in range(lo, hi):
            c2, j = divmod(ci, NK)
            if True:
                pq = pmm.tile([128, 512], F32, name="pq", tag="pmm", bufs=4)
                for sub in range(2):
                    t0 = b * N + 512 * c2 + 256 * sub
                    dst = pq[:, 256 * sub:256 * (sub + 1)]
                    for a in range(4):
                        nc.tensor.matmul(dst, wqfj[:, j, 0, 2 * a:2 * a + 2, :],
                                         XF[:, 1, 2 * a:2 * a + 2, t0:t0 + 256],
                                         start=(a == 0), stop=False, perf_mode=DR)
                    nc.tensor.matmul(dst, wqfj[:, j, :, 8, :],
                                     XF[:, :, 8, t0:t0 + 256],
                                     start=False, stop=False, perf_mode=DR)
                    for k in range(8):
                        nc.tensor.matmul(dst, wqfj[:, j, :, k, :],
                                         XF[:, :, k, t0:t0 + 256],
                                         start=False, stop=False, perf_mode=DR)
                    nc.tensor.matmul(dst, wq8r[:, :, 128 * j:128 * (j + 1)],
                                     XF[:, :, 8, t0:t0 + 256],
                                     start=False, stop=(sub == 1), perf_mode=DR)
                u = wp.tile([128, 512], BF16, name="u", tag="u", bufs=2)
                nc.scalar.activation(u[:], pq[:], AF.Relu, scale=1.0 / SW,
                                     bias=cn.wqb[:, j:j + 1])
                u2 = wp.tile([128, 512], BF16, name="u2", tag="u2", bufs=2)
                col = j * CH + c2
                nc.scalar.activation(u2[:], u[:], AF.Square,
                                     accum_out=st.acc1q[:, col:col + 1])
                q3s = st.q3[j][:, 512 * c2:512 * (c2 + 1)]
                nc.vector.tensor_mul(q3s, u2[:], u[:])
                u6 = wp.tile([128, 512], BF16, name="u6", tag="u6", bufs=2)
                nc.vector.tensor_mul(u6[:], q3s, q3s)
                nc.vector.tensor_reduce(st.acc2q[:, col:col + 1], u6[:],
                                        axis=AX.X, op=ALU.add)

    def ph_dwc(hook):
        st.vdwc = [wp.tile([96, N], BF16, name=f"vdwc_{g}", tag=f"vdwc_{g}")
                   for g in range(KVH)]
        for g in range(KVH):
            if hook is not None:
                hook(g)
            v3 = st.vT8[g][:].rearrange("p (y x) -> p y x", y=32)
            for hf in range(2):
                pd = pmm.tile([96, 512], F32, name="pd", tag="pdw", bufs=2)
                p3 = pd[:].rearrange("p (y x) -> p y x", y=16)
                mms = []
                for dxi, dx in enumerate((-1, 0, 1)):
                    x0, x1 = max(0, -dx), 32 - max(0, dx)
                    # dy=0 tap paired with a zero-weight slot -> DoubleRow rate
                    base = v3[0:96, 16 * hf:16 * hf + 16, x0 + dx:x1 + dx]
                    mms.append((cn.dgz[:, :, g, dxi, :], _ins_dim(base, 0, 2),
                                p3[:, 0:16, x0:x1], DR))
                ya0 = max(1, 16 * hf)
                ya1 = min(31, 16 * hf + 16)
                for dxi, dx in enumerate((-1, 0, 1)):
                    x0, x1 = max(0, -dx), 32 - max(0, dx)
                    base = v3[0:96, ya0 - 1:ya1 - 1, x0 + dx:x1 + dx]
                    rhs = _ins_dim(base, 64, 2)
                    mms.append((cn.dgp[:, :, g, dxi, :], rhs,
                                p3[:, ya0 - 16 * hf:ya1 - 16 * hf, x0:x1], DR))
                for dxi, dx in enumerate((-1, 0, 1)):
                    x0, x1 = max(0, -dx), 32 - max(0, dx)
                    if hf == 0:  # y=0, tap dy=+1
                        mms.append((cn.dge[:, g, 6 + dxi, :],
                                    v3[0:96, 1:2, x0 + dx:x1 + dx],
                                    p3[:, 0:1, x0:x1], None))
                    else:        # y=31, tap dy=-1
                        mms.append((cn.dge[:, g, dxi, :],
                                    v3[0:96, 30:31, x0 + dx:x1 + dx],
                                    p3[:, 15:16, x0:x1], None))
                for mi, (lhsT, rhs, out, pm) in enumerate(mms):
                    nc.tensor.matmul(out, lhsT, rhs, start=(mi == 0),
                                     stop=(mi == len(mms) - 1), perf_mode=pm)
                nc.scalar.activation(st.vdwc[g][:, 512 * hf:512 * (hf + 1)],
                                     pd[:], AF.Identity, scale=1.0 / SW,
                                     bias=cn.dwcb[:, g:g + 1])

    def ph_norms():
        sq_rows = []
        for ai, acc in enumerate((st.acc1q, st.acc2q)):
            accs = wp.tile([128, NK], F32, name="accs", tag="accs", bufs=2)
            av = acc[:, 0:NK * CH].rearrange("p (j c) -> p j c", c=CH)
            nc.vector.tensor_add(accs[:], av[:, :, 0], av[:, :, 1])
            accsb = wp.tile([128, NK], BF16, name="accsb", tag="accsb", bufs=2)
            nc.vector.tensor_copy(accsb[:], accs[:])
            psn = pa.tile([1, H], F32, name="psn", tag="pa", bufs=2)
            for j in range(NK):
                nc.tensor.matmul(psn[:], accsb[:, j:j + 1], cn.masks[:, j, :],
                                 start=(j == 0), stop=(j == NK - 1))
            srow = wp.tile([1, H], F32, name="srow", tag="srow", bufs=4)
            nc.vector.tensor_copy(srow[:], psn[:])
            sq_rows.append(srow)
        acc2kb = wp.tile([128, KVH * TT], BF16, name="acc2kb", tag="acc2kb",
                         bufs=2)
        nc.vector.tensor_copy(acc2kb[:], st.acc2k[:])
        psk = pa.tile([1, KVH * TT], F32, name="psk", tag="pa", bufs=2)
        nc.tensor.matmul(psk[:], cn.ones_c[:], acc2kb[:], start=True, stop=True)
        krow = wp.tile([1, KVH * TT], F32, name="krow", tag="krow", bufs=2)
        nc.vector.tensor_copy(krow[:], psk[:])
        kred2 = wp.tile([1, KVH], F32, name="kred2", tag="kred2", bufs=2)
        nc.vector.tensor_reduce(kred2[:],
                                krow[:].rearrange("a (k t) -> a k t", k=KVH),
                                axis=AX.X, op=ALU.add)
        sk_rows = [st.kred1, kred2]

        def _f_row(s1, s2, width, tagp):
            se = wp.tile([1, width], F32, name="se", tag=f"se{tagp}", bufs=2)
            nc.vector.tensor_scalar_add(se[:], s2[:], 1e-30)
            rc = wp.tile([1, width], F32, name="rc", tag=f"rc{tagp}", bufs=2)
            nc.vector.reciprocal(rc[:], se[:])
            rt = wp.tile([1, width], F32, name="rt", tag=f"rt{tagp}", bufs=2)
            nc.vector.tensor_mul(rt[:], s1[:], rc[:])
            fr = wp.tile([1, width], F32, name="fr", tag=f"fr{tagp}", bufs=2)
            nc.scalar.activation(fr[:], rt[:], AF.Sqrt)
            return fr

        fq = _f_row(sq_rows[0], sq_rows[1], H, "q")
        fk = _f_row(sk_rows[0], sk_rows[1], KVH, "k")
        fk12 = wp.tile([1, H], F32, name="fk12", tag="fk12", bufs=2)
        for g in range(3):
            nc.vector.tensor_copy(fk12[:, 4 * g:4 * (g + 1)], fk[:])
        grow = wp.tile([1, H], F32, name="grow", tag="grow", bufs=2)
        nc.vector.tensor_mul(grow[:], fq[:], fk12[:])
        gb = wp.tile([96, H], F32, name="gb", tag="gb", bufs=2)
        nc.gpsimd.partition_broadcast(gb[:], grow[:], channels=96)
        st.kvp = [wp.tile([96, 96], BF16, name=f"kvp_{h}", tag=f"kvp_{h}")
                  for h in range(H)]
        for h in range(H):
            nc.vector.tensor_scalar_mul(st.kvp[h][:], st.kvu[h % KVH][:],
                                        gb[:, h:h + 1])

    def ph_e2(c2, filler=None):
        if c2 == 0:
            st.OT = [wp.tile([128, N], BF16, name=f"OT_{j}", tag=f"OT_{j}")
                     for j in range(NK)]
        for h in range(H):
            if filler is not None:
                fn = next(filler, None)
                if fn is not None:
                    fn()
            pieces = _head_pieces(h)
            if len(pieces) == 1:
                j0, r00, _, _ = pieces[0]
                rhs = st.q3[j0][r00:r00 + 96, 512 * c2:512 * (c2 + 1)]
            else:
                qh = wp.tile([96, 512], BF16, name="qh", tag="qh", bufs=4)
                for pi, (j, r0, rr, cnt) in enumerate(pieces):
                    src_ap = st.q3[j][r0:r0 + cnt, 512 * c2:512 * (c2 + 1)]
                    if (h + pi) % 2 == 0:
                        nc.vector.tensor_copy(qh[rr:rr + cnt, :], src_ap)
                    else:
                        nc.scalar.copy(qh[rr:rr + cnt, :], src_ap)
                rhs = qh[:]
            pa_t = pa.tile([96, 512], F32, name="pat", tag="pa", bufs=2)
            nc.tensor.matmul(pa_t[:], st.kvp[h][:], rhs, start=True,
                             stop=True)
            if len(pieces) == 1:
                j0, r00, _, _ = pieces[0]
                nc.vector.tensor_tensor(
                    out=st.OT[j0][r00:r00 + 96, 512 * c2:512 * (c2 + 1)],
                    in0=pa_t[:],
                    in1=st.vdwc[h % KVH][:, 512 * c2:512 * (c2 + 1)],
                    op=ALU.add)
            else:
                pac = wp.tile([96, 512], BF16, name="pac", tag="pac", bufs=4)
                nc.scalar.copy(pac[:], pa_t[:])
                for (j, r0, rr, cnt) in pieces:
                    nc.vector.tensor_tensor(
                        out=st.OT[j][r0:r0 + cnt, 512 * c2:512 * (c2 + 1)],
                        in0=pac[rr:rr + cnt, :],
                        in1=st.vdwc[h % KVH][rr:rr + cnt,
                                             512 * c2:512 * (c2 + 1)],
                        op=ALU.add)

    def _proj_group(t, oc):
        py = pmm.tile([128, 384], F32, name="py", tag="pdw", bufs=2)
        for j in range(NK):
            nc.tensor.matmul(py[:], st.OT[j][:, 128 * t:128 * (t + 1)],
                             cn.PWT[j][:, 384 * oc:384 * (oc + 1)],
                             start=(j == 0), stop=(j == NK - 1))
        ysb = wp.tile([128, 384], F32, name="ysb", tag="ysb", bufs=3)
        # bias rides the psum evacuation (pjb_bc pre-broadcast on host)
        nc.vector.tensor_tensor(out=ysb[:], in0=py[:],
                                in1=cn.pjb_bc[:, 384 * oc:384 * (oc + 1)],
                                op=ALU.add)
        t0 = b * N + 128 * t
        nc.sync.dma_start(out=cn.y_out[t0:t0 + 128, 384 * oc:384 * (oc + 1)],
                          in_=ysb[:])

    def ph_proj_groups(ta, tb):
        return [(lambda t=t, oc=oc: _proj_group(t, oc))
                for t in range(ta, tb) for oc in range(3)]

    def ph_proj(ta, tb):
        for fn in ph_proj_groups(ta, tb):
            fn()

    return dict(k_begin=ph_k_begin, k_tile=ph_k_tile, v_tile=ph_v_tile,
                v_end=ph_v_end, vt=ph_vt, e1=ph_e1, q=ph_q,
                dwc=ph_dwc, norms=ph_norms, e2=ph_e2, proj=ph_proj,
                proj_groups=ph_proj_groups)


_NC_CACHE = None


def _get_nc():
    global _NC_CACHE
    if _NC_CACHE is None:
        _NC_CACHE = _build_kernel()
    return _NC_CACHE


def _hi_lo(a):
    hi = a.astype(_F8)
    lo = (a - hi.astype(np.float32)).astype(_F8)
    return hi, lo


def _host_consts(wq_w, wq_b, wkv_w, wkv_b, dwc_w, dwc_b, proj_w, proj_b):
    wqT = np.ascontiguousarray(np.asarray(wq_w, np.float32).T) * SW      # [in, out]
    wkvT = np.ascontiguousarray(np.asarray(wkv_w, np.float32).T) * SW    # [in, 768]
    qhi, qlo = _hi_lo(wqT)
    khi, klo = _hi_lo(wkvT)

    # wqfj: [128, j, slot(hi,lo), k, 128]
    wqfj = np.zeros((128, NK, 2, NK, 128), _F8)
    for k in range(NK):
        for j in range(NK):
            wqfj[:, j, 0, k, :] = qhi[128 * k:128 * (k + 1), 128 * j:128 * (j + 1)]
            wqfj[:, j, 1, k, :] = qlo[128 * k:128 * (k + 1), 128 * j:128 * (j + 1)]
    wq8r = np.zeros((128, 2, DIM), _F8)
    wq8r[:, 0, :] = qlo[128 * 8:, :]
    wq8r[:, 1, :] = qhi[128 * 8:, :]

    wkvf = np.zeros((128, 2, NK, 768), _F8)
    for k in range(NK):
        wkvf[:, 0, k, :] = khi[128 * k:128 * (k + 1), :]
        wkvf[:, 1, k, :] = klo[128 * k:128 * (k + 1), :]
    wkv8r = np.zeros((128, 2, 768), _F8)
    wkv8r[:, 0, :] = klo[128 * 8:, :]
    wkv8r[:, 1, :] = khi[128 * 8:, :]
    wkvfk = np.ascontiguousarray(wkvf[:, :, :, 0:384])
    wkvfv = np.ascontiguousarray(wkvf[:, :, :, 384:768])
    wkv8rk = np.ascontiguousarray(wkv8r[:, :, 0:384])
    wkv8rv = np.ascontiguousarray(wkv8r[:, :, 384:768])

    pwT = np.ascontiguousarray(np.asarray(proj_w, np.float32).T).astype(_BF)
    wqb = np.ascontiguousarray(np.asarray(wq_b, np.float32).reshape(NK, 128).T)
    kvb64 = (np.asarray(wkv_b, np.float32).reshape(1, 768) * SW).astype(_BF)
    pjb_bc = np.broadcast_to(np.asarray(proj_b, np.float32).reshape(1, DIM),
                             (128, DIM)).astype(_BF)

    dw = np.asarray(dwc_w, np.float32).reshape(KVH, 96, 9) * SW  # [g, d, tap]
    dgp = np.zeros((96, 2, KVH, 3, 96), np.float32)
    dgz = np.zeros((96, 2, KVH, 3, 96), np.float32)
    dge = np.zeros((96, KVH, 9, 96), np.float32)
    for d in range(96):
        for dxi in range(3):
            dgp[d, 0, :, dxi, d] = dw[:, d, 0 + dxi]       # dy=-1 taps 0,1,2
            dgp[d, 1, :, dxi, d] = dw[:, d, 6 + dxi]       # dy=+1 taps 6,7,8
            dgz[d, 0, :, dxi, d] = dw[:, d, 3 + dxi]       # dy=0 taps, slot1=0
        for ti in range(9):
            dge[d, :, ti, d] = dw[:, d, ti]
    dgp = dgp.astype(_F8)
    dgz = dgz.astype(_F8)
    dge = dge.astype(_F8)
    dwcb = np.ascontiguousarray(np.asarray(dwc_b, np.float32).reshape(KVH, 96).T)

    mk = np.zeros((128, NK, H), np.float32)
    for j in range(NK):
        for p in range(128):
            f = 128 * j + p
            mk[p, j, f // 96] = 1.0
    masks = mk.astype(_BF)
    eye = np.eye(128, dtype=np.float32).astype(_BF)
    return dict(wqfj=wqfj, wq8r=wq8r, wkvfk=wkvfk, wkvfv=wkvfv, wkv8rk=wkv8rk,
                wkv8rv=wkv8rv, pwT=pwT, wqb=wqb, kvb64=kvb64, pjb_bc=pjb_bc,
                dgp=dgp, dgz=dgz, dge=dge, dwcb=dwcb, masks=masks, eye=eye)


def kernel(x, wq_w, wq_b, wkv_w, wkv_b, dwc_w, dwc_b, proj_w, proj_b,
           _want_results=False, **_unused):
    nc = _get_nc()
    consts = _host_consts(wq_w, wq_b, wkv_w, wkv_b, dwc_w, dwc_b, proj_w, proj_b)
    x = np.asarray(x, np.float32)
    in_maps = []
    for c in range(NCORES):
        m = dict(consts)
        m["x"] = np.ascontiguousarray(x[BL * c:BL * (c + 1)].reshape(T, DIM))
        in_maps.append(m)
    res = bass_utils.run_bass_kernel_spmd(nc, in_maps, core_ids=list(range(NCORES)))
    y = np.stack([res.results[c]["y"].reshape(BL, N, DIM) for c in range(NCORES)])
    y = y.reshape(B, N, DIM)
    if _want_results:
        return y, res
    return y
